# revision 50
# baseline (speedup 1.0000x reference)
"""GAT forward (2-layer graph attention, B=4 N=2048 F=128 H=8 D=64 C=32)
as a Bass/Tile SPMD kernel on 8 Trainium2 NeuronCores.

Sharding: core c -> (batch b=c//2, query-row half c%2).  Each core computes
attention for its 1024 query rows over all 2048 keys for all 8 heads
(layer 1) and for the output head (layer 2).  The only cross-core exchange
is a 2-rank AllGather of the layer-2 projections [g1|g2|Wh2] ([1024,34] f32)
within each (2b, 2b+1) pair.

Layout: attention logits are built TRANSPOSED, e^T[j (keys) = partitions,
i (queries) = free], so the PV matmul needs no operand transposes
(lhsT = Wh[j,d] stationary, rhs = p[j,i] moving, out = h^T[d,i]) and
softmax row sums come from a ones-column appended to Wh (PSUM row D).

The N^2 elementwise work over 144 [128,1024] logit tiles is the bottleneck.
Key identity: after a softmax-invariant per-query rescale by exp(-.2 f1_i),
  exp(prelu(f1_i + f2_j)) = max(exp(.8 f1_i)*exp(f2_j), exp(.2 f2_j))
and the adjacency mask is applied MULTIPLICATIVELY (m01 in {0,1}), so a
pair of logit tiles [128,2,1024] needs only:
  P7 : q = ts(B1, e1c, e2c, mult, max)  per sub   (DVE tensor-scalar 4x!)
       p = q * m01                      per pair  (DVE tensor-tensor 2x)
  P5 : same q on DVE; p = q * m01 on GPSIMD (tensor_mult)
  P3 : u = Prelu(f1rep + f2col) (ACT)  p' = Exp(u) (ACT)  p = p'*m01 (DVE)
  P4 : same ACT ops; the m01 multiply runs on GPSIMD
where B1 = exp(.8 f1_i) replicated, e1c = exp(f2_j), e2c = exp(.2 f2_j)
per-partition f32 scalars.  Dual-scalar tensor_scalar and the bf16
tensor ops give DVE 2-4x element rates; routes are chosen per pair to
jointly saturate DVE+ACT+GPSIMD (cost-model hill-climb).

Softmax division is deferred to the per-head-pair epilogue (1/S via DVE
reciprocal), fused with ELU via elu(v)+1 = relu(v) + exp(min(v,0)),
the +1 folded into a rank-1 correction matmul of the layer-2 projection.
"""

import numpy as np
import ml_dtypes

import concourse.bass as bass
import concourse.tile as tile
from concourse import mybir
from concourse.bass_utils import run_bass_kernel_spmd

F32 = mybir.dt.float32
F32R = mybir.dt.float32r
BF16 = mybir.dt.bfloat16

B, N, F, H, D, C = 4, 2048, 128, 8, 64, 32
I = N // 2          # query rows per core
JT = N // 128       # key tiles
IC = I // 128       # query-row 128-chunks per core
KT = (H * D) // 128 # hidden-dim 128-chunks
ALPHA = 0.2
BIG = 1e15          # mask bias; also dominates A*B in the factored path
N_CORES = 8
REPLICA_GROUPS = [[0, 1], [2, 3], [4, 5], [6, 7]]

ADD = mybir.AluOpType.add
MAX = mybir.AluOpType.max
MULT = mybir.AluOpType.mult
# NOTE: hardware "Lrelu" has a fixed 0.01 slope and ignores alpha;
# "Prelu" honors alpha (verified on HW) — it is the configurable leaky relu.
ACT_LRELU = mybir.ActivationFunctionType.Prelu
ACT_EXP = mybir.ActivationFunctionType.Exp
ACT_LN = mybir.ActivationFunctionType.Ln
ACT_COPY = mybir.ActivationFunctionType.Copy
ACT_RECIP = mybir.ActivationFunctionType.Reciprocal

N_HEADS_ALL = H + 1          # 8 layer-1 heads + the layer-2 output head
PAIRS_PER_HEAD = JT // 2     # 8
N_PAIRS = N_HEADS_ALL * PAIRS_PER_HEAD  # 72


def _split_multiwaits(nc):
    """Pinned walrus accepts only one sync-wait per instruction; Tile's exit
    drain (and occasionally others) carries several.  Hoist extras onto
    single-wait Drains on the same engine immediately before the owner."""
    n_fixed = 0
    for fn in nc.m.functions:
        for bb in fn.blocks:
            for name in [i.name for i in bb.instructions]:
                idx = [i.name for i in bb.instructions].index(name)
                inst = bb.instructions[idx]
                si = inst.sync_info
                if si is None or len(si.on_wait) <= 1:
                    continue
                waits = list(si.on_wait)
                for k, w in enumerate(waits[:-1]):
                    nd = mybir.InstDrain(
                        name=f"waitfix-{inst.name}-{k}", ins=[], outs=[])
                    nd.engine = inst.engine
                    nd.sync_info = mybir.SyncInfo(on_wait=[w], on_update=[])
                    nc.register_instruction(nd, overwrite=True)
                    bb.instructions.insert(idx + k, nd)
                inst.sync_info = mybir.SyncInfo(
                    on_wait=waits[-1:], on_update=list(si.on_update))
                n_fixed += 1
    return n_fixed


def _make_route(cfg):
    """Per-pair pipeline assignment:
      7 (DVE ts-dual + DVE m01-mult),   5 (DVE ts-dual + GPS m01-mult),
      3 (ACT prelu/exp + DVE m01-mult), 4 (ACT prelu/exp + GPS m01-mult).
    Counts spread across the 9 'heads' (8 L1 + L2)."""
    if "route" in cfg:                      # explicit per-pair override
        route = list(cfg["route"])
        assert len(route) == N_PAIRS
        return route
    # counts apply to the 64 layer-1 pairs; layer 2 is its own l2route
    n4 = int(cfg.get("p4", 0))
    n7 = int(cfg.get("p7", 17))
    n5 = int(cfg.get("p5", 25))
    n3 = H * PAIRS_PER_HEAD - n4 - n7 - n5
    assert n3 >= 0
    l2r = list(cfg.get("l2route", [5, 7, 7, 5, 7, 7, 7, 7]))
    assert len(l2r) == PAIRS_PER_HEAD
    # one largest-remainder interleave over all 64 L1 slots, chunked into
    # heads: every head gets a balanced, alternating mode mix
    want = {7: n7, 5: n5, 3: n3, 4: n4}
    nslots = H * PAIRS_PER_HEAD
    acc = {m: 0.0 for m in want}
    route = []
    for _ in range(nslots):
        for m in want:
            acc[m] += want[m]
        pick = max(want, key=lambda m: (acc[m], want[m]))
        acc[pick] -= nslots
        route.append(pick)
    route.extend(l2r)
    return route


def build_program(with_collective=True, cfg=None, repeat=1):
    cfg = dict(cfg or {})
    QB[0] = int(cfg.get("qbufs", 3))
    route = _make_route(cfg)

    nc = bass.Bass("TRN2", target_bir_lowering=False, debug=False,
                   enable_asserts=False, num_devices=N_CORES)

    xt_d = nc.dram_tensor("xt", [F, N], BF16, kind="ExternalInput")
    xtl_d = nc.dram_tensor("xtl", [F, I], BF16, kind="ExternalInput")
    mb_d = nc.dram_tensor("mb", [JT, 128, I], BF16, kind="ExternalInput")
    wext_d = nc.dram_tensor("wext", [H, F, D + 2], BF16, kind="ExternalInput")
    a1rep_d = nc.dram_tensor("a1rep", [H, F, 128], BF16, kind="ExternalInput")
    woext_d = nc.dram_tensor("woext", [KT, 128, C + 2], F32,
                             kind="ExternalInput")
    wcorr_d = nc.dram_tensor("wcorr", [1, C + 2], F32, kind="ExternalInput")
    ident_d = nc.dram_tensor("ident", [128, 128], F32, kind="ExternalInput")
    outp_d = nc.dram_tensor("outp", [C, I], F32, kind="ExternalOutput")

    with tile.TileContext(nc) as tc:
        if repeat > 1:
            def body(iv, unroll=None):
                _build_body(nc, tc, xt_d, xtl_d, mb_d, wext_d, a1rep_d,
                            woext_d, wcorr_d, ident_d, outp_d,
                            with_collective, route, cfg)
            with tc.For_i(0, repeat, 1) as iv:
                body(iv)
        else:
            _build_body(nc, tc, xt_d, xtl_d, mb_d, wext_d, a1rep_d,
                        woext_d, wcorr_d, ident_d, outp_d,
                        with_collective, route, cfg)
    _split_multiwaits(nc)
    return nc


QB = [3]


def _emit_pair(nc, work, workp, mode, subs, pair_args, mpair):
    """Emit one logit pair.  subs = [(jt, m01_ap, f2col_ap, e1col_ap,
    e2col_ap)]; pair_args = (f1rep, B1); mpair = [128,2,I] m01 view."""
    f1rep, B1 = pair_args
    if mode in (5, 7):
        q = work.tile([128, 2, I], BF16, tag="q1", bufs=QB[0])
        for k, (jt, m_ap, f2c, e1c, e2c) in enumerate(subs):
            # q = max(exp(.8 f1_i)*exp(f2_j), exp(.2 f2_j))  [one 4x TS op]
            nc.vector.tensor_scalar(q[:, k, :], B1[:], e1c, e2c, MULT, MAX)
        p = workp.tile([128, 2, I], BF16, tag="p")
        if mode == 5:
            for k in range(2):
                nc.gpsimd.tensor_mul(p[:, k, :], q[:, k, :], mpair[:, k, :])
        else:
            nc.vector.tensor_tensor(out=p[:], in0=q[:], in1=mpair, op=MULT)
        return p
    # ACT pipelines (3: DVE mask-mult, 4: GPS mask-mult)
    u = work.tile([128, 2, I], BF16, tag="u")
    for k, (jt, m_ap, f2c, e1c, e2c) in enumerate(subs):
        nc.scalar.activation(u[:, k, :], f1rep[:], ACT_LRELU,
                             bias=f2c, alpha=ALPHA)
    nc.scalar.activation(u[:], u[:], ACT_EXP)
    p = workp.tile([128, 2, I], BF16, tag="p")
    if mode == 4:
        for k in range(2):
            nc.gpsimd.tensor_mul(p[:, k, :], u[:, k, :], mpair[:, k, :])
    else:
        nc.vector.tensor_tensor(out=p[:], in0=u[:], in1=mpair, op=MULT)
    return p


def _copy_engine(nc, eng, out, in_):
    if eng == "act":
        nc.scalar.activation(out, in_, ACT_COPY)
    elif eng == "gps":
        nc.gpsimd.tensor_copy(out=out, in_=in_)
    else:
        nc.vector.tensor_copy(out=out, in_=in_)


def _build_body(nc, tc, xt_d, xtl_d, mb_d, wext_d, a1rep_d, woext_d,
                wcorr_d, ident_d, outp_d, with_collective, route, cfg):
    from contextlib import ExitStack
    ctx = ExitStack()
    f1rep_eng = cfg.get("f1rep_eng", "dve")
    rbc_eng = cfg.get("rbc_eng", "act")
    fcol_eng = cfg.get("fcol_eng", "dve")
    rinv_eng = cfg.get("rinv_eng", "dve")
    stt_eng = cfg.get("stt_eng", "dve")
    assert f1rep_eng != "gps" and rbc_eng != "gps"  # GPSIMD cannot read PSUM
    
    ph0_engs = cfg.get("ph0_engs", ("act", "dve"))
    ep_v_gps = False  # GPSIMD cannot read PSUM (hT)
    with ctx:
        singles = ctx.enter_context(tc.tile_pool(name="singles", bufs=1))
        psA = ctx.enter_context(tc.tile_pool(
            name="psA", bufs=int(cfg.get("psa", 2)), space="PSUM"))
        psB = ctx.enter_context(tc.tile_pool(
            name="psB", bufs=int(cfg.get("psb", 1)), space="PSUM"))
        psC = ctx.enter_context(tc.tile_pool(name="psC", bufs=2, space="PSUM"))
        dram = ctx.enter_context(tc.tile_pool(name="dram", bufs=1,
                                              space="DRAM"))

        # ---------------- persistent loads ----------------
        mb_s = singles.tile([128, JT, I], BF16)
        xtl_s = singles.tile([F, I], BF16)
        nc.sync.dma_start(out=xtl_s[:], in_=xtl_d.ap())
        a1rep_s = singles.tile([F, H, 128], BF16)
        nc.sync.dma_start(out=a1rep_s[:],
                          in_=a1rep_d.ap().rearrange("h f e -> f h e"))
        wcorr_s = singles.tile([1, C + 2], F32)
        nc.sync.dma_start(out=wcorr_s[:], in_=wcorr_d.ap())
        ident_s = singles.tile([128, 128], F32)
        nc.sync.dma_start(out=ident_s[:], in_=ident_d.ap())
        woext_raw = singles.tile([128, KT, C + 2], F32)
        nc.sync.dma_start(out=woext_raw[:],
                          in_=woext_d.ap().rearrange("k f e -> f k e"))
        woext_s = singles.tile([128, KT, C + 2], F32R)
        nc.vector.tensor_copy(out=woext_s[:], in_=woext_raw[:])

        ones_s = singles.tile([1, 128], BF16)
        nc.gpsimd.memset(ones_s[:], 1.0)
        onesf_s = singles.tile([1, 128], F32)
        nc.gpsimd.memset(onesf_s[:], 1.0)

        whbuf = singles.tile([128, H, JT, D + 1], BF16)
        nc.gpsimd.memset(whbuf[:, :, :, D:D + 1], 1.0)
        fcol = singles.tile([128, H, JT, 1], F32)
        fexp1 = singles.tile([128, H, JT, 1], F32)
        fexp2 = singles.tile([128, H, JT, 1], F32)
        hcatT = singles.tile([128, KT, I], F32R)

        xt_s = singles.tile([F, N], BF16)
        nc.sync.dma_start(out=xt_s[:], in_=xt_d.ap())
        wext_s = singles.tile([F, H, D + 2], BF16)
        nc.sync.dma_start(out=wext_s[:],
                          in_=wext_d.ap().rearrange("h f e -> f h e"))
        nc.sync.dma_start(out=mb_s[:, 0:2, :],
                          in_=mb_d.ap()[0:2].rearrange("jt p i -> p jt i"))
        for j0 in (2, 6, 10):
            j1 = j0 + 4 if j0 < 10 else JT
            nc.sync.dma_start(
                out=mb_s[:, j0:j1, :],
                in_=mb_d.ap()[j0:j1].rearrange("jt p i -> p jt i"))

        work = ctx.enter_context(
            tc.tile_pool(name="work", bufs=int(cfg.get("wbufs", 4))))
        workp = ctx.enter_context(
            tc.tile_pool(name="workp", bufs=int(cfg.get("pbufs", 4))))
        ep1 = ctx.enter_context(tc.tile_pool(name="ep1", bufs=1))
        ep2 = ctx.enter_context(tc.tile_pool(name="ep2", bufs=2))
        epL2 = ctx.enter_context(tc.tile_pool(name="epL2", bufs=1))

        def emit_phase0_head(h):
            # Wh tiles + f columns for head h (emitted per-head so the
            # copies overlap the previous head's logit work)
            for jg in range(JT // 4):
                whp = psA.tile([128, 4, D + 2], F32, tag="ph")
                for k in range(4):
                    jt = jg * 4 + k
                    nc.tensor.matmul(whp[:, k, :],
                                     lhsT=xt_s[:, jt * 128:(jt + 1) * 128],
                                     rhs=wext_s[:, h, :])
                dst = whbuf[:, h, jg * 4:(jg + 1) * 4, 0:D]
                _copy_engine(nc, ph0_engs[jg % len(ph0_engs)],
                             dst, whp[:, :, 0:D])
                _copy_engine(nc, fcol_eng,
                             fcol[:, h, jg * 4:(jg + 1) * 4, :],
                             whp[:, :, D + 1:D + 2])
            nc.scalar.activation(fexp1[:, h], fcol[:, h], ACT_EXP)
            nc.scalar.activation(fexp2[:, h], fcol[:, h], ACT_EXP,
                                 scale=ALPHA)

        ep_state = {}

        def _stt_hcat(out_ap, v_ap, t_ap):
            eng = nc.gpsimd if stt_eng == "gps" else nc.vector
            eng.scalar_tensor_tensor(out=out_ap, in0=v_ap, scalar=0.0,
                                     in1=t_ap, op0=MAX, op1=ADD)

        def emit_half_ep(hT, h, sliced=False):
            # per-head half-epilogue: rinv = 1/S via DVE reciprocal, PE
            # partition broadcast, v-half = hT*rinv.  The odd half finishes:
            # hcat = elu(v)+1 = relu(v)+exp(min(v,0)).  `sliced` pipelines
            # the chain in 512-column slices (used for the final head-pair,
            # where this chain gates the whole layer-2 tail).
            rinv = ep1.tile([1, I], F32, tag=f"ri{h % 2}", bufs=1)
            if h % 2 == 0:
                v = ep1.tile([128, I], BF16, tag="v", bufs=2)
                ep_state["v"] = v
                half = slice(0, D)
            else:
                v = ep_state["v"]
                half = slice(D, 128)
            rbp = psB.tile([128, I], F32, tag="rep")
            # HW: a DVE op may read only ONE input from PSUM, so the
            # broadcast row block is staged through SBUF (rbc)
            rbc = ep1.tile([D, I], F32, tag=f"rb{h % 2}", bufs=1)
            t = None
            if h % 2 == 1:
                t = ep1.tile([128, I], BF16, tag="t", bufs=2)
            eslices = [slice(k_ * 512, (k_ + 1) * 512)
                       for k_ in range(I // 512)]
            for sl_ in eslices:
                nc.vector.reciprocal(rinv[0:1, sl_], hT[D:D + 1, sl_])
                nc.tensor.matmul(rbp[0:D, sl_], lhsT=onesf_s[0:1, 0:D],
                                 rhs=rinv[0:1, sl_])
            if sliced:
                # stage-major across slices: each engine pipelines
                for sl_ in eslices:
                    _copy_engine(nc, rbc_eng, rbc[:, sl_], rbp[0:D, sl_])
                for sl_ in eslices:
                    nc.vector.tensor_tensor(out=v[half, sl_],
                                            in0=hT[0:D, sl_],
                                            in1=rbc[:, sl_], op=MULT)
                    if h % 2 == 1:
                        nc.vector.tensor_scalar_min(t[:, sl_], v[:, sl_], 0.0)
                if h % 2 == 1:
                    for sl_ in eslices:
                        nc.scalar.activation(t[:, sl_], t[:, sl_], ACT_EXP)
                        _stt_hcat(hcatT[:, h // 2, sl_], v[:, sl_], t[:, sl_])
            if not sliced:
                if cfg.get("rbc_dma", 0):
                    # PSUM->SBUF broadcast copy via DMA: no engine time,
                    # latency hidden by the deferred-epilogue window
                    nc.sync.dma_start(out=rbc[:], in_=rbp[0:D, :])
                else:
                    _copy_engine(nc, rbc_eng, rbc[:], rbp[0:D, :])
                nc.vector.tensor_tensor(out=v[half, :], in0=hT[0:D, :],
                                        in1=rbc[:], op=MULT)
                if h % 2 == 1:
                    nc.vector.tensor_scalar_min(t[:], v[:], 0.0)
                    nc.scalar.activation(t[:], t[:], ACT_EXP)
                    _stt_hcat(hcatT[:, h // 2, :], v[:], t[:])

        ph0_done = set()

        def emit_phase0_once(h):
            if h not in ph0_done:
                ph0_done.add(h)
                emit_phase0_head(h)

        def emit_head_prep(h):
            emit_phase0_once(h)
            head_modes = route[h * PAIRS_PER_HEAD:(h + 1) * PAIRS_PER_HEAD]
            need_f1rep = any(m in (3, 4) for m in head_modes)
            need_B = any(m in (5, 7) for m in head_modes)
            f1p = psB.tile([128, I], F32, tag="rep")
            for hf in range(I // 512):
                sl = slice(hf * 512, (hf + 1) * 512)
                nc.tensor.matmul(f1p[:, sl], lhsT=a1rep_s[:, h, :],
                                 rhs=xtl_s[:, sl])
            f1rep_s = B1_s = None
            if need_f1rep:
                f1rep_s = ep2.tile([128, I], BF16, tag="f1rep")
                _copy_engine(nc, f1rep_eng, f1rep_s[:], f1p[:])
            if need_B:
                B1_s = ep2.tile([128, I], BF16, tag="B1")
                nc.scalar.activation(B1_s[:], f1p[:], ACT_EXP, scale=1.0 - ALPHA)
            return f1rep_s, B1_s

        # ---------------- layer 1 ----------------
        # prefetch phase-0 (Wh/fcol) for the first heads: PE + copies run
        # under the startup mask-DMA window (head 0 first)
        for h0_ in range(0, 1 + int(cfg.get("prefetch_heads", 1))):
            emit_phase0_once(h0_)
        pending_ep = None   # deferred half-epilogue (software pipelining)
        preps = emit_head_prep(0)
        for h in range(H):
            head_modes = route[h * PAIRS_PER_HEAD:(h + 1) * PAIRS_PER_HEAD]
            cur = preps
            hT = psC.tile([D + 1, I], F32, tag="acc")
            for jp in range(PAIRS_PER_HEAD):
                if jp == int(cfg.get("ep_defer", 2)) and pending_ep is not None:
                    pending_ep()
                    pending_ep = None
                if jp == int(cfg.get("prep_at", 4)) and h + 1 < H:
                    preps = emit_head_prep(h + 1)
                mode = head_modes[jp]
                subs = []
                for k in range(2):
                    jt = jp * 2 + k
                    subs.append((jt, mb_s[:, jt, :],
                                 fcol[:, h, jt, :], fexp1[:, h, jt, :],
                                 fexp2[:, h, jt, :]))
                p = _emit_pair(nc, work, workp, mode, subs, cur,
                               mb_s[:, jp * 2:jp * 2 + 2, :])
                for k in range(2):
                    jt = jp * 2 + k
                    for hf in range(I // 512):
                        sl = slice(hf * 512, (hf + 1) * 512)
                        nc.tensor.matmul(hT[:, sl],
                                         lhsT=whbuf[:, h, jt, :],
                                         rhs=p[:, k, sl],
                                         start=(jt == 0), stop=(jt == JT - 1))

            pending_ep = (lambda t_=hT, h_=h, s_=(h == H - 1):
                          emit_half_ep(t_, h_, sliced=s_))
        if pending_ep is not None:
            pending_ep()
            pending_ep = None

        # ---------------- layer 2 projection + exchange -------------------
        # HOST permutes the key order per core to [my I queries; partner's I
        # queries], so key tiles jt 0..7 are LOCAL (read straight from
        # wh2loc, no collective round-trip) and only tiles 8..15 need the
        # partner's projection.  The exchange is an AllReduce(add) of the
        # local projection; partner = sum - mine (exact to f32 rounding).
        wh2loc = singles.tile([128, IC, C + 2], F32)
        gin = dram.tile([I, C + 2], F32)
        g1rowp = psB.tile([128, I], F32, tag="rep")
        for ic in range(IC):
            w2p = psA.tile([128, 4, D + 2], F32, tag="ph")
            for kt in range(KT):
                nc.tensor.matmul(
                    w2p[:, 0, 0:C + 2],
                    lhsT=hcatT[:, kt, ic * 128:(ic + 1) * 128],
                    rhs=woext_s[:, kt, :],
                    start=(kt == 0), stop=False)
            nc.tensor.matmul(w2p[:, 0, 0:C + 2], lhsT=onesf_s[0:1, :],
                             rhs=wcorr_s[:], start=False, stop=True)
            nc.vector.tensor_copy(out=wh2loc[:, ic, :], in_=w2p[:, 0, 0:C + 2])
            nc.tensor.transpose(g1rowp[0:1, ic * 128:(ic + 1) * 128],
                                in_=wh2loc[:, ic, 0:1], identity=ident_s[:])
        gsum = dram.tile([I, C + 2], F32)
        # exchange pipelined in two halves: gin-half DMA -> AllReduce-half
        # (or local fake copy) -> wh2sum-half DMA
        for hf in range(2):
            ghalf = slice(hf * 512, (hf + 1) * 512)
            nc.sync.dma_start(
                out=gin[ghalf, :].rearrange("(ic p) c -> p ic c", p=128),
                in_=wh2loc[:, hf * 4:(hf + 1) * 4, :])
            if with_collective:
                nc.gpsimd.collective_compute(
                    "AllReduce", mybir.AluOpType.add,
                    replica_groups=REPLICA_GROUPS,
                    ins=[gin[ghalf, :].opt()], outs=[gsum[ghalf, :].opt()])
            else:  # timing-model variant: fake the exchange, local copy
                nc.sync.dma_start(out=gsum[ghalf, :], in_=gin[ghalf, :])

        # g1 row (local queries) -> replicated [128, I]
        g1row_s = epL2.tile([1, I], BF16, tag="g1row")
        nc.scalar.activation(g1row_s[:], g1rowp[0:1, :], ACT_COPY)
        g1rp = psB.tile([128, I], F32, tag="rep")
        for hf in range(I // 512):
            sl = slice(hf * 512, (hf + 1) * 512)
            nc.tensor.matmul(g1rp[:, sl], lhsT=ones_s[0:1, :],
                             rhs=g1row_s[0:1, sl])
        l2_modes = route[H * PAIRS_PER_HEAD:]
        g1rep_s = B1L2 = None
        if any(m in (3, 4) for m in l2_modes):
            g1rep_s = singles.tile([128, I], BF16)
            nc.vector.tensor_copy(out=g1rep_s[:], in_=g1rp[:])
        if any(m in (5, 7) for m in l2_modes):
            B1L2 = singles.tile([128, I], BF16)
            nc.scalar.activation(B1L2[:], g1rp[:], ACT_EXP, scale=1.0 - ALPHA)

        # key-side rows: [g1, g2, Wh2(32)] f32, bf16 for the PV lhsT.
        # Local tiles (jt 0..7) come straight from wh2loc; remote tiles
        # (jt 8..15) from the AllReduce sum minus the local projection.
        JH = JT // 2
        wh2gr = singles.tile([128, JT, C + 3], BF16)
        nc.gpsimd.memset(wh2gr[:, :, C + 2:C + 3], 1.0)
        its1 = singles.tile([128, JT, 1], F32)
        its2 = singles.tile([128, JT, 1], F32)
        for jg in range(JH // 4):
            s4 = slice(jg * 4, (jg + 1) * 4)
            nc.gpsimd.tensor_copy(out=wh2gr[:, s4, 0:C + 2],
                                  in_=wh2loc[:, s4, :])
            nc.scalar.activation(its1[:, s4, :], wh2loc[:, s4, 1:2], ACT_EXP)
            nc.scalar.activation(its2[:, s4, :], wh2loc[:, s4, 1:2], ACT_EXP,
                                 scale=ALPHA)
        wh2sum = singles.tile([128, JH, C + 2], F32)
        wh2rem = singles.tile([128, JH, C + 2], F32)
        for jg in range(JH // 4):
            s4 = slice(jg * 4, (jg + 1) * 4)
            s4r = slice(JH + jg * 4, JH + (jg + 1) * 4)
            nc.sync.dma_start(
                out=wh2sum[:, s4, :],
                in_=gsum[jg * 512:(jg + 1) * 512, :].rearrange(
                    "(jt p) c -> p jt c", p=128))
            nc.vector.tensor_tensor(out=wh2rem[:, s4, :],
                                    in0=wh2sum[:, s4, :], in1=wh2loc[:, s4, :],
                                    op=mybir.AluOpType.subtract)
            nc.gpsimd.tensor_copy(out=wh2gr[:, s4r, 0:C + 2],
                                  in_=wh2rem[:, s4, :])
            nc.scalar.activation(its1[:, s4r, :], wh2rem[:, s4, 1:2], ACT_EXP)
            nc.scalar.activation(its2[:, s4r, :], wh2rem[:, s4, 1:2], ACT_EXP,
                                 scale=ALPHA)

        # ---------------- layer 2 attention ----------------
        o2T = psC.tile([D + 1, I], F32, tag="acc")
        for jp in range(PAIRS_PER_HEAD):
            mode = l2_modes[jp]
            subs = []
            for k in range(2):
                jt = jp * 2 + k
                f2c = (wh2loc[:, jt, 1:2] if jt < JH
                       else wh2rem[:, jt - JH, 1:2])
                subs.append((jt, mb_s[:, jt, :],
                             f2c, its1[:, jt, :],
                             its2[:, jt, :]))
            p = _emit_pair(nc, work, workp, mode, subs,
                           (g1rep_s, B1L2), mb_s[:, jp * 2:jp * 2 + 2, :])
            for k in range(2):
                jt = jp * 2 + k
                for hf in range(I // 512):
                    sl = slice(hf * 512, (hf + 1) * 512)
                    nc.tensor.matmul(o2T[0:C + 1, sl],
                                     lhsT=wh2gr[:, jt, 2:C + 3],
                                     rhs=p[:, k, sl],
                                     start=(jt == 0), stop=(jt == JT - 1))

        # ---------------- finalize (transposed: per-query reciprocal) -----
        # output leaves the device TRANSPOSED [C, I]; the host unshard
        # transposes back (free on CPU), so no PE transpose round-trip
        rinv2 = epL2.tile([1, I], F32, tag="ri2")
        oT_s = epL2.tile([C, I], F32, tag="oT")
        rbc2p = psB.tile([128, I], F32, tag="rep")
        rbc2_s = epL2.tile([C, I], F32, tag="rbc2")
        NSL = 4
        W2 = I // NSL
        sls = [slice(hf * W2, (hf + 1) * W2) for hf in range(NSL)]
        for sl in sls:                       # stage-major: engines pipeline
            nc.vector.reciprocal(rinv2[0:1, sl], o2T[C:C + 1, sl])
            nc.tensor.matmul(rbc2p[0:C, sl], lhsT=onesf_s[0:1, 0:C],
                             rhs=rinv2[0:1, sl])
        for sl in sls:
            _copy_engine(nc, rbc_eng, rbc2_s[:, sl], rbc2p[0:C, sl])
        for hf in range(2):
            sl = slice(hf * 512, (hf + 1) * 512)
            nc.vector.tensor_tensor(out=oT_s[:, sl], in0=o2T[0:C, sl],
                                    in1=rbc2_s[:, sl], op=MULT)
            nc.sync.dma_start(out=outp_d.ap()[:, sl], in_=oT_s[:, sl])


# --------------------------------------------------------------------------
# host side
# --------------------------------------------------------------------------

def shard_inputs(x, adj, W, a1, a2, Wo, ao1, ao2):
    x = np.asarray(x, np.float32)
    adj = np.asarray(adj)
    W = np.asarray(W, np.float32)
    a1 = np.asarray(a1, np.float32)
    a2 = np.asarray(a2, np.float32)
    Wo = np.asarray(Wo, np.float32)
    ao1 = np.asarray(ao1, np.float32)
    ao2 = np.asarray(ao2, np.float32)
    BF = ml_dtypes.bfloat16

    wvec1 = np.einsum("hfd,hd->hf", W, a1)          # [H, F]
    wvec2 = np.einsum("hfd,hd->hf", W, a2)
    wext = np.concatenate([W, wvec1[:, :, None], wvec2[:, :, None]],
                          axis=2).astype(BF)
    a1rep = np.repeat(wvec1[:, :, None], 128, axis=2).astype(BF)
    wo1 = Wo @ ao1                                   # [512]
    wo2 = Wo @ ao2
    woflat = np.concatenate([wo1[:, None], wo2[:, None], Wo], 1)  # [512, 34]
    woext = woflat.reshape(KT, 128, C + 2).astype(np.float32)
    wcorr = (-woflat.sum(0))[None, :].astype(np.float32)
    ident = np.eye(128, dtype=np.float32)

    in_maps = []
    for c in range(N_CORES):
        b, half = c // 2, c % 2
        i0 = half * I
        # Per-core key order [my I queries; partner's I queries]: key tiles
        # jt 0..7 are then LOCAL in layer 2 (see kernel L2 exchange).
        # Attention is key-order invariant as long as xt and the mask rows
        # are permuted consistently.
        perm = np.r_[i0:i0 + I, (I - i0):(I - i0) + I]
        xt = np.ascontiguousarray(x[b].T[:, perm]).astype(BF)   # [F, N]
        xtl = np.ascontiguousarray(xt[:, 0:I])
        adjt = adj[b, i0:i0 + I, :].T[perm, :]       # [N, I] = (j, i)
        mb = np.where(adjt > 0, np.float32(1.0), np.float32(0.0))
        mb = np.ascontiguousarray(mb.reshape(JT, 128, I)).astype(BF)
        in_maps.append({
            "xt": xt, "xtl": xtl, "mb": mb, "wext": wext,
            "a1rep": a1rep, "woext": woext, "wcorr": wcorr, "ident": ident,
        })
    return in_maps


# Engine routing: multiplicative-mask pipelines balanced across
# DVE (P7) / DVE+GPS (P5) / ACT+DVE (P3) / ACT+GPS (P4) by cost-model
# (TimelineSim) hill-climb.
DEFAULT_CFG = {"ep_defer": 2, "f1rep_eng": "act", "rbc_eng": "act",
               "prep_at": 4, "p4": 0, "p7": 20, "p5": 24,
               "l2route": [7, 5, 7, 7, 5, 7, 7, 7],
               "pbufs": 8, "wbufs": 6, "qbufs": 5, "prefetch_heads": 1}

_CACHE = {}


def _program():
    if "nc" not in _CACHE:
        _CACHE["nc"] = build_program(with_collective=True, cfg=DEFAULT_CFG)
    return _CACHE["nc"]


def kernel(**inputs):
    nc = _program()
    in_maps = shard_inputs(**inputs)
    res = run_bass_kernel_spmd(nc, in_maps, list(range(N_CORES)))
    _CACHE["last_results"] = res
    out = np.empty((B, N, C), np.float32)
    for c in range(N_CORES):
        b, half = c // 2, c % 2
        out[b, half * I:(half + 1) * I, :] = res.results[c]["outp"].T
    return out

